# revision 33
# baseline (speedup 1.0000x reference)
"""PilotNet (quantized, brevitas-style fake-quant) forward pass on 8 TRN2 cores.

Strategy
--------
Pure data parallel: 256 images -> 8 cores x 32 images. Weights replicated.

Numerics: the network is a fixed-point lattice. QuantReLU outputs are
a_int * 0.4 with a_int in {0..15}; fake-quant weights are w_int / 7 with
w_int in {-7..7}. All of conv2..fc3 is computed in the *integer domain*
(activations a_int stored as fp8e4m3 ints, weights w_int as fp8e4m3 ints,
TensorE accumulates exact integer sums in fp32 PSUM), which is bit-exact.
Layer boundary: next_int = min(round((acc + b_int)/7), 15) after relu.
round() is RNE via the +2^23 trick on the vector engine; (acc+b_int)/7 is
never within fp32 error of a .5 boundary (distance >= 1/14), so exact.
conv1 consumes the raw fp32 input and runs as an fp32 matmul.

Conv lowering (design "a"): K = (kx, ci) on partitions using x-replicated
resident staging planes (slot (kx,ci) holds act[ci, row, s*x+kx]); the
row taps r = s*dy + ky accumulate in PSUM over r-tiles; M = (dy, co) packs
R output rows to keep PE and DVE utilization high.
"""
import numpy as np
import ml_dtypes

FP8 = ml_dtypes.float8_e4m3
C_RNE = 8388608.0          # 2^23 magic round constant
INV7 = float(np.float32(1.0) / np.float32(7.0))
B_SCALE = float(np.float32(1.0 / 7.0) * np.float32(0.4))
N_CORES = 8
IMG_PER_CORE = 32
NIMG = 16                  # images per chunk
NCHUNK = 2

# ---------------------------------------------------------------------------
# host-side weight preprocessing (ints in [-7,7], biases in int units)
# ---------------------------------------------------------------------------

def _w_int(w):
    return np.clip(np.round(np.asarray(w, np.float32) * 7.0), -7, 7).astype(np.float32)

def _b_int(b):
    return np.clip(np.round(np.asarray(b, np.float32) / np.float32(B_SCALE)),
                   -128, 127).astype(np.float32)

def _build_lhsT1(w1):
    w1i = _w_int(w1)                 # integer weights: exact in fp32r (fp22)
    lhsT = np.zeros((85, 128), np.float32)
    for r in range(17):
        for kx in range(5):
            for dy in range(5):
                ky = r - 3 * dy
                if 0 <= ky < 5:
                    lhsT[kx * 17 + r, dy * 24:(dy + 1) * 24] = w1i[:, 0, ky, kx]
    return lhsT

def _build_conv_lhsT(wi, R, stride, kxa):
    co, ci, k, _ = wi.shape
    nr = k + stride * (R - 1)
    def mk(kxs):
        out = np.zeros((len(kxs) * ci, nr, R * co), np.float32)
        for i, kx in enumerate(kxs):
            for c in range(ci):
                for r in range(nr):
                    for dy in range(R):
                        ky = r - stride * dy
                        if 0 <= ky < k:
                            out[i * ci + c, r, dy * co:(dy + 1) * co] = wi[:, c, ky, kx]
        return out
    return mk(list(range(kxa))), (mk(list(range(kxa, k))) if kxa < k else None)

def _build_fc1_lhsT(fw1):
    wi = _w_int(fw1)  # [100, 16640]
    out = np.zeros((128, 130, 100), np.float32)
    for p in range(128):
        q, c5 = divmod(p, 64)
        for t in range(130):
            h, x2 = divmod(t, 10)
            out[p, t] = wi[:, c5 * 260 + h * 20 + (2 * x2 + q)]
    return out

def _pad_m(a, m=128):
    # pad lhsT last dim (M columns) with zeros to 128 for FWL eligibility
    if a.shape[-1] < m:
        pad = [(0, 0)] * (a.ndim - 1) + [(0, m - a.shape[-1])]
        a = np.pad(a, pad)
    return a


def preprocess(inp):
    d = {}
    d['w1t'] = _build_lhsT1(inp['w1'])
    w2t, _ = _build_conv_lhsT(_w_int(inp['w2']), 3, 2, 5)             # [120,9,108]
    d['w2t'] = _pad_m(w2t)                                            # [120,9,128]
    w3A, w3B = _build_conv_lhsT(_w_int(inp['w3']), 2, 2, 3)
    d['w3At'], d['w3Bt'] = _pad_m(w3A), _pad_m(w3B)                   # [*,7,128]
    d['w4At'], d['w4Bt'] = _build_conv_lhsT(_w_int(inp['w4']), 2, 1, 2)
    d['w5At'], d['w5Bt'] = _build_conv_lhsT(_w_int(inp['w5']), 2, 1, 2)
    d['fcw1t'] = _build_fc1_lhsT(inp['fw1'])                          # [128,130,100]
    d['fcw2t'] = _w_int(inp['fw2']).T.copy()                          # [100,50]
    d['fcw3t'] = _w_int(inp['fw3']).T.copy()                          # [50,10]
    d['owt'] = _w_int(inp['ow']).T.copy()                             # [10,1]
    b1q = (_b_int(inp['b1']) * np.float32(B_SCALE)).astype(np.float32)
    d['b1t'] = np.tile(b1q * np.float32(2.5), 5).reshape(120, 1)
    d['b2t'] = np.pad(np.tile(_b_int(inp['b2']) * np.float32(INV7), 3),
                      (0, 20)).reshape(128, 1)
    d['b3t'] = np.pad(np.tile(_b_int(inp['b3']) * np.float32(INV7), 2),
                      (0, 32)).reshape(128, 1)
    d['b4t'] = np.tile(_b_int(inp['b4']) * np.float32(INV7), 2).reshape(128, 1)
    d['b5t'] = np.tile(_b_int(inp['b5']) * np.float32(INV7), 2).reshape(128, 1)
    d['bf1t'] = (_b_int(inp['fb1']) * np.float32(INV7)).reshape(100, 1)
    d['bf2t'] = (_b_int(inp['fb2']) * np.float32(INV7)).reshape(50, 1)
    d['bf3t'] = (_b_int(inp['fb3']) * np.float32(INV7)).reshape(10, 1)
    ob_int = float(_b_int(inp['ob'])[0])
    fp8_keys = {'w2t', 'w3At', 'w3Bt', 'w4At', 'w4Bt', 'w5At', 'w5Bt',
                'fcw1t', 'fcw2t', 'fcw3t', 'owt'}
    out = {}
    for k, v in d.items():
        out[k] = np.ascontiguousarray(v.astype(FP8) if k in fp8_keys
                                      else v.astype(np.float32))
    return out, ob_int


# ---------------------------------------------------------------------------
# bass kernel builder
# ---------------------------------------------------------------------------
# Layout conventions (all act/staging tiles, img innermost so (x,img) DMA
# dims merge contiguously; DMA APs are limited to 3 dims, last contiguous):
#   xph   dram [32, 3, 240, 107] f32: x phase-split along width (stride 3)
#   a1t   [120=(dy,co24), 16g, 2ph, 53x', 16img] f8   row=5g+dy, x=2x'+ph
#   repl2 [120=(kx,ci24), 79row, 52x, 16img] f8       slot=act1[ci,row,2x+kx]
#   a2t   [108=(dy,co36), 13g, 2ph, 26x', 16img] f8   row=3g+dy, x=2x'+ph (<51)
#   repl3 A[108=(kx012,ci36)|B 72, 38row, 24x, 16img] f8
#   a3t   [96=(dy,co48), 9g, 24x, 16img] f8           row=2g+dy
#   repl4 A[96=(kx01,ci48)|B 48, 17row, 22x, 16img] f8
#   a4t   [128=(dy,co64), 8g, 22x, 16img] f8          row=2g+dy
#   repl5 A[128=(kx01,ci64)|B 64, 15row, 20x, 16img] f8
#   a5t   [128=(dy,co64), 7g, 2ph, 10x'', 16img] f8   row=2g+dy, x=2x''+ph
#   f1r   [128=(q,c5_64), 2chunk, 130t, 16img] f8     feat=c5*260+h*20+2x2+q

def build_bass(ob_int, debug=False, n_layers=9):
    import concourse.bass as bass
    import concourse.bacc as bacc
    import concourse.mybir as mybir
    import concourse.tile as tile
    from contextlib import ExitStack

    dt = mybir.dt
    F32, F8 = dt.float32, dt.float8e4
    Relu = mybir.ActivationFunctionType.Relu
    Alu = mybir.AluOpType
    DR = mybir.MatmulPerfMode.DoubleRow

    nc = bacc.Bacc("TRN2", target_bir_lowering=False, debug=False)

    F32R = dt.float32r
    # xr1: host-restaged conv1 rhs. Row (c*16+g) holds [85, 2, 16, 106]:
    # hi/lo 12-bit-mantissa split of x (slot (kx*17+r, lvl, img, xo) from
    # x[img, 15g+r, 3*xo+kx]); contiguous 13568 B per partition, one DMA.
    # fp32r matmuls with integer weights keep every product exact.
    xt = nc.dram_tensor("xr1", [NCHUNK * 16, 85, 2 * NIMG * 106], F32R,
                        kind="ExternalInput")
    yt = nc.dram_tensor("y", [IMG_PER_CORE, 1], F32, kind="ExternalOutput")
    wspec = dict(
        w1t=([85, 128], F32R), w2t=([120, 9, 128], F8),
        w3At=([108, 7, 128], F8), w3Bt=([72, 7, 128], F8),
        w4At=([96, 4, 128], F8), w4Bt=([48, 4, 128], F8),
        w5At=([128, 4, 128], F8), w5Bt=([64, 4, 128], F8),
        fcw1t=([128, 130, 100], F8), fcw2t=([100, 50], F8),
        fcw3t=([50, 10], F8), owt=([10, 1], F8),
        b1t=([120, 1], F32), b2t=([128, 1], F32), b3t=([128, 1], F32),
        b4t=([128, 1], F32), b5t=([128, 1], F32),
        bf1t=([100, 1], F32), bf2t=([50, 1], F32), bf3t=([10, 1], F32),
    )
    wdram = {k: nc.dram_tensor(k, shp, d, kind="ExternalInput")
             for k, (shp, d) in wspec.items()}
    dbg = {}
    if debug:
        for nm, shp in dict(a1=[120, 16, 2, 53, 16], a2=[108, 13, 2, 26, 16],
                            a3=[96, 9, 24, 16], a4=[128, 8, 22, 16],
                            a5=[128, 7, 2, 10, 16], f1r=[128, 2, 130, 16],
                            af1=[100, 32], af3=[10, 32]).items():
            dbg[nm] = nc.dram_tensor("dbg_" + nm, shp, F8, kind="ExternalOutput")

    with tile.TileContext(nc) as tc, ExitStack() as ctx:
        wpool = ctx.enter_context(tc.tile_pool(name="weights", bufs=1))
        w = {}
        for k, (shp, d) in wspec.items():
            w[k] = wpool.tile(shp, d, tag=k, name="w_" + k)
            nc.scalar.dma_start(w[k][:], wdram[k].ap())

        a1t = wpool.tile([120, 16, 2, 53, 16], F8, tag="a1t")
        repl2 = wpool.tile([120, 79, 52, 16], F8, tag="repl2")
        a2t = wpool.tile([108, 13, 2, 26, 16], F8, tag="a2t")
        repl3A = wpool.tile([108, 38, 24, 16], F8, tag="repl3A")
        repl3B = wpool.tile([72, 38, 24, 16], F8, tag="repl3B")
        a3t = wpool.tile([96, 9, 24, 16], F8, tag="a3t")
        repl4A = wpool.tile([96, 17, 22, 16], F8, tag="repl4A")
        repl4B = wpool.tile([48, 17, 22, 16], F8, tag="repl4B")
        a4t = wpool.tile([128, 8, 22, 16], F8, tag="a4t")
        repl5A = wpool.tile([128, 15, 20, 16], F8, tag="repl5A")
        repl5B = wpool.tile([64, 15, 20, 16], F8, tag="repl5B")
        a5t = wpool.tile([128, 7, 2, 10, 16], F8, tag="a5t")
        f1r = wpool.tile([128, 2, 130, 16], F8, tag="f1r")

        r1pool = ctx.enter_context(tc.tile_pool(name="r1", bufs=2))
        qpool = ctx.enter_context(tc.tile_pool(name="qtmp", bufs=3))
        p1pool = ctx.enter_context(tc.tile_pool(name="p1", bufs=3, space="PSUM"))
        p2pool = ctx.enter_context(tc.tile_pool(name="p2", bufs=4, space="PSUM"))
        pfpool = ctx.enter_context(tc.tile_pool(name="pf", bufs=1, space="PSUM"))

        # zero the padding column of repl2 (x=51) so matmul never sees junk
        nc.gpsimd.memset(repl2[:, :, 51, :], 0)
        if debug:  # unwritten partial-group corners otherwise trip the sim
            for t in (a1t, a2t, a3t, a4t, a5t):
                nc.gpsimd.memset(t[:], 0)

        def conv_taps(psum, wt, repl, row0, nr, first, last, xsl=slice(None)):
            # accumulate nr row taps into psum: DoubleRow pairs + odd tail
            insts = [(r, 2) for r in range(0, nr - 1, 2)]
            if nr % 2:
                insts.append((nr - 1, 1))
            for j, (r, wd) in enumerate(insts):
                st = first and j == 0
                sp = last and j == len(insts) - 1
                if wd == 2:
                    nc.tensor.matmul(psum, wt[:, r:r + 2, :],
                                     repl[:, row0 + r:row0 + r + 2, xsl, :],
                                     start=st, stop=sp, perf_mode=DR)
                else:
                    nc.tensor.matmul(psum, wt[:, r, :],
                                     repl[:, row0 + r, xsl, :],
                                     start=st, stop=sp)

        def quant(psum_ap, bias_ap, scale, out_ap, P, F):
            t1 = qpool.tile([P, F], F32, tag="q1", name="q1")
            t2 = qpool.tile([P, F], F32, tag="q2", name="q2")
            nc.scalar.activation(t1[:], psum_ap, Relu, bias=bias_ap, scale=scale)
            nc.vector.tensor_scalar_add(t2[:], t1[:], C_RNE)
            nc.vector.tensor_scalar(out=out_ap, in0=t2[:], scalar1=C_RNE,
                                    scalar2=15.0, op0=Alu.subtract, op1=Alu.min)

        # ---- per-chunk building blocks (emission order = pipeline order) ----
        SC1 = float(np.float32(2.5) * np.float32(INV7))
        def conv1_group(c, g):
            M = 120 if g < 15 else 96
            for h in range(2):          # 8-image halves (SBUF pressure)
                r1 = r1pool.tile([85, 2, 8, 106], F32R, tag="r1", name="r1")
                src = bass.AP(xt, (c * 16 + g) * 85 * 3392 + h * 848,
                              [[3392, 85], [1696, 2], [1, 848]])
                nc.sync.dma_start(r1[:, :, :, :], src)
                for qh in range(2):
                    q = h * 2 + qh
                    p1 = p1pool.tile([128, 4, 106], F32, tag="p1", name="p1")
                    for lvl in range(2):
                        nc.tensor.matmul(p1[:, :, :], w["w1t"][:, :],
                                         r1[:, lvl, qh * 4:(qh + 1) * 4, :],
                                         start=(lvl == 0), stop=(lvl == 1))
                    out_ap = a1t[0:M, g, :, :, q * 4:(q + 1) * 4] \
                        .transpose([0, 3, 2, 1])
                    quant(p1[0:M, :, :].rearrange("p a b -> p (a b)"),
                          w["b1t"][0:M, :], SC1, out_ap, M, 424)

        def stage2(glo, ghi, eng=None):   # repl2 rows for a1 groups [glo, ghi)
            eng = eng or nc.gpsimd
            for kx in range(5):
                ph, qx = kx % 2, kx // 2
                for dy in range(5):
                    gc = 16 if dy < 4 else 15
                    hi = min(ghi, gc)
                    if hi <= glo:
                        continue
                    eng.dma_start(
                        repl2[kx * 24:(kx + 1) * 24,
                              dy + 5 * glo:dy + 5 * (hi - 1) + 1:5, 0:51, :],
                        a1t[dy * 24:(dy + 1) * 24, glo:hi, ph, qx:qx + 51, :])

        def conv2_c(c, post):
            for y3 in range(13):
                R = 3 if y3 < 12 else 2
                M, nr = 36 * R, 2 * R + 3
                p2h = [p2pool.tile([128, 26, 16], F32, tag="p2", name="p2")
                       for _ in range(2)]
                for xh in range(2):
                    conv_taps(p2h[xh][:, :, :], w["w2t"], repl2, 6 * y3, nr,
                              True, True, xsl=slice(26 * xh, 26 * xh + 26))
                for xh in range(2):
                    out_ap = a2t[0:M, y3, :, 13 * xh:13 * xh + 13, :] \
                        .transpose([0, 2, 1, 3])
                    quant(p2h[xh][0:M, :, :].rearrange("p a b -> p (a b)"),
                          w["b2t"][0:M, :], INV7, out_ap, M, 416)
                if y3 in post:
                    post[y3]()

        def stage3(glo, ghi):       # repl3 rows for a2 groups (y3) [glo, ghi)
            for kx in range(5):
                ph, qx = kx % 2, kx // 2
                for dy in range(3):
                    gc = len(range(dy, 38, 3))
                    hi = min(ghi, gc)
                    if hi <= glo:
                        continue
                    dst = repl3A if kx < 3 else repl3B
                    kxo = kx if kx < 3 else kx - 3
                    nc.gpsimd.dma_start(
                        dst[kxo * 36:(kxo + 1) * 36,
                            dy + 3 * glo:dy + 3 * (hi - 1) + 1:3, :, :],
                        a2t[dy * 36:(dy + 1) * 36, glo:hi, ph, qx:qx + 24, :])

        def conv3_c(c, post):
            for y2 in range(9):
                R = 2 if y2 < 8 else 1
                M, nr = 48 * R, 2 * R + 3
                p3 = p2pool.tile([128, 24, 16], F32, tag="p2", name="p3")
                conv_taps(p3[:, :, :], w["w3At"], repl3A, 4 * y2, nr, True, False)
                conv_taps(p3[:, :, :], w["w3Bt"], repl3B, 4 * y2, nr, False, True)
                quant(p3[0:M, :, :].rearrange("p a b -> p (a b)"),
                      w["b3t"][0:M, :], INV7,
                      a3t[0:M, y2, :, :].rearrange("p a b -> p (a b)"), M, 384)
                if y2 in post:
                    post[y2]()

        def stage4(glo, ghi):       # repl4 rows for a3 groups (y2) [glo, ghi)
            for kx in range(3):
                for dy in range(2):
                    gc = len(range(dy, 17, 2))
                    hi = min(ghi, gc)
                    if hi <= glo:
                        continue
                    dst = repl4A if kx < 2 else repl4B
                    kxo = kx if kx < 2 else kx - 2
                    nc.gpsimd.dma_start(
                        dst[kxo * 48:(kxo + 1) * 48,
                            dy + 2 * glo:dy + 2 * (hi - 1) + 1:2, :, :],
                        a3t[dy * 48:(dy + 1) * 48, glo:hi, kx:kx + 22, :])

        def conv4_c(c, post):
            for t in range(8):
                R = 2 if t < 7 else 1
                M, nr = 64 * R, R + 2
                p4 = p2pool.tile([128, 22, 16], F32, tag="p2", name="p4")
                conv_taps(p4[:, :, :], w["w4At"], repl4A, 2 * t, nr, True, False)
                conv_taps(p4[:, :, :], w["w4Bt"], repl4B, 2 * t, nr, False, True)
                quant(p4[0:M, :, :].rearrange("p a b -> p (a b)"),
                      w["b4t"][0:M, :], INV7,
                      a4t[0:M, t, :, :].rearrange("p a b -> p (a b)"), M, 352)
                if t in post:
                    post[t]()

        def stage5(glo, ghi):       # repl5 rows for a4 groups (t) [glo, ghi)
            for kx in range(3):
                for dy in range(2):
                    gc = len(range(dy, 15, 2))
                    hi = min(ghi, gc)
                    if hi <= glo:
                        continue
                    dst = repl5A if kx < 2 else repl5B
                    kxo = kx if kx < 2 else kx - 2
                    nc.gpsimd.dma_start(
                        dst[kxo * 64:(kxo + 1) * 64,
                            dy + 2 * glo:dy + 2 * (hi - 1) + 1:2, :, :],
                        a4t[dy * 64:(dy + 1) * 64, glo:hi, kx:kx + 20, :])

        def conv5_c(c):
            for t in range(7):
                R = 2 if t < 6 else 1
                M, nr = 64 * R, R + 2
                p5 = p2pool.tile([128, 20, 16], F32, tag="p2", name="p5")
                conv_taps(p5[:, :, :], w["w5At"], repl5A, 2 * t, nr, True, False)
                conv_taps(p5[:, :, :], w["w5Bt"], repl5B, 2 * t, nr, False, True)
                out_ap = a5t[0:M, t, :, :, :].transpose([0, 2, 1, 3])
                quant(p5[0:M, :, :].rearrange("p a b -> p (a b)"),
                      w["b5t"][0:M, :], INV7, out_ap, M, 320)

        def f1r_stage(c):
            f1r_t = f1r[:].tensor
            for dy in range(2):
                for q in range(2):
                    gc = len(range(dy, 13, 2))
                    dst = bass.AP(f1r_t,
                                  (q * 64) * (2 * 130 * 16) + c * (130 * 16)
                                  + (10 * dy) * 16,
                                  [[2 * 130 * 16, 64], [20 * 16, gc], [1, 160]])
                    nc.sync.dma_start(dst, a5t[dy * 64:(dy + 1) * 64, 0:gc, q, :, :])

        def conv2to5(c, after3=None, after4=None):
            conv2_c(c, {6: lambda: stage3(0, 7)})
            stage3(7, 13)
            if after3:
                after3()
            conv3_c(c, {4: lambda: stage4(0, 5)})
            stage4(5, 9)
            if after4:
                after4()
            conv4_c(c, {3: lambda: stage5(0, 4)})
            stage5(4, 8)
            conv5_c(c)
            f1r_stage(c)

        # ---- pipelined schedule: chunk1 conv1 overlaps chunk0 staging ----
        for g in range(8):
            conv1_group(0, g)
        stage2(0, 8)
        for g in range(8, 16):
            conv1_group(0, g)
        stage2(8, 16)
        for g in range(16):
            conv1_group(1, g)
        # chunk-1 repl2 staging interleaves into the gpsimd queue between
        # chunk-0's stage3/stage4 windows; transfers overlap chunk-0 conv3/4
        conv2to5(0, after3=lambda: stage2(0, 8), after4=lambda: stage2(8, 16))
        conv2to5(1)
        pf1 = pfpool.tile([100, 2, 16], F32, tag="pf", name="pf1")
        for t in range(130):
            nc.tensor.matmul(pf1[:], w["fcw1t"][:, t, :], f1r[:, :, t, :],
                             start=(t == 0), stop=(t == 129))
        af1 = wpool.tile([100, 32], F8, tag="af1")
        quant(pf1[:].rearrange("p a b -> p (a b)"), w["bf1t"][:], INV7,
              af1[:], 100, 32)
        pf2 = pfpool.tile([50, 32], F32, tag="pf", name="pf2")
        nc.tensor.matmul(pf2[:], w["fcw2t"][:], af1[:], start=True, stop=True)
        af2 = wpool.tile([50, 32], F8, tag="af2")
        quant(pf2[:], w["bf2t"][:], INV7, af2[:], 50, 32)
        pf3 = pfpool.tile([10, 32], F32, tag="pf", name="pf3")
        nc.tensor.matmul(pf3[:], w["fcw3t"][:], af2[:], start=True, stop=True)
        af3 = wpool.tile([10, 32], F8, tag="af3")
        quant(pf3[:], w["bf3t"][:], INV7, af3[:], 10, 32)
        pout = pfpool.tile([1, 32], F32, tag="pf", name="pout")
        nc.tensor.matmul(pout[:], w["owt"][:], af3[:], start=True, stop=True)
        yout = wpool.tile([1, 32], F32, tag="yout")
        nc.vector.tensor_scalar(out=yout[:], in0=pout[:], scalar1=ob_int,
                                scalar2=B_SCALE, op0=Alu.add, op1=Alu.mult)
        nc.sync.dma_start(yt.ap(), yout[:])

        if debug:
            for nm, t in [("a1", a1t), ("a2", a2t), ("a3", a3t), ("a4", a4t),
                          ("a5", a5t), ("f1r", f1r), ("af1", af1), ("af3", af3)]:
                nc.sync.dma_start(dbg[nm].ap(), t[:])

    nc.compile()
    return nc


# ---------------------------------------------------------------------------
# entry point
# ---------------------------------------------------------------------------

def build_xr1(x):
    """[32,1,240,320] -> [32 planes, 85, 2, 16*106] conv1 rhs staging planes.

    Plane (c*16+g), slot (kx*17+r, lvl, img, xo) from x[c*16+img, 0, 15g+r,
    3*xo+kx], split exactly into hi (top 12 mantissa bits) + lo halves so
    fp32r matmuls with integer weights stay product-exact."""
    out = np.zeros((NCHUNK, 16, 5, 17, 2, NIMG, 106), np.float32)
    xs = x[:, 0]                                     # [32, 240, 320]
    for kx in range(5):
        sub = np.ascontiguousarray(xs[:, :, kx:kx + 316:3])   # [32, 240, 106]
        hi = (sub.view(np.uint32) & np.uint32(0xFFFFF000)).view(np.float32)
        lo = sub - hi                                # exact, <= 12 sig bits
        for c in range(NCHUNK):
            s = slice(c * NIMG, (c + 1) * NIMG)
            for g in range(16):
                nr = min(17, 240 - 15 * g)
                rs = slice(15 * g, 15 * g + nr)
                out[c, g, kx, :nr, 0] = hi[s, rs].transpose(1, 0, 2)
                out[c, g, kx, :nr, 1] = lo[s, rs].transpose(1, 0, 2)
    return out.reshape(NCHUNK * 16, 85, 2 * NIMG * 106)


def kernel(**inputs):
    from concourse import bass_utils
    x = np.ascontiguousarray(np.asarray(inputs['x'], np.float32))
    wmaps, ob_int = preprocess(inputs)
    nc = build_bass(ob_int)
    in_maps = []
    for core in range(N_CORES):
        m = {'xr1': build_xr1(x[core * 32:(core + 1) * 32])}
        m.update(wmaps)
        in_maps.append(m)
    res = bass_utils.run_bass_kernel_spmd(nc, in_maps, core_ids=list(range(N_CORES)))
    out = np.concatenate([res.results[i]['y'] for i in range(N_CORES)], axis=0)
    return out.astype(np.float32)

